# revision 24
# baseline (speedup 1.0000x reference)
"""Multi-head cross-attention (B=2, Q=KV=2048, H=1024, 16 heads x 64) on 8
Trainium2 NeuronCores via Bass/Tile.

Sharding: core c handles batch b = c//4 and head group g = c%4 (4 heads, 256
hidden columns). Every core is fully independent (no collectives): it
computes its heads' q/k/v projections, RoPE, masked scores, softmax, context,
and a partial dense projection (its 256 rows of Wd). The host sums the 4
dense partials per batch and adds bias + residual, and transposes the
q/k/context outputs that the device produces in PE-friendly layouts.

Device-side structure:
  - hidden/encoder states arrive pre-transposed (hidT/encT [H, S]); q^T and
    k^T projections are computed directly (weights stationary, hidT moving),
    so no PE transposes are needed on the input side.
  - mask + 1/sqrt(d) scale are folded into the score matmul: stationary is
    q_rot^T augmented with a row of ones, moving is k_rot^T augmented with a
    row of where(mask, 0, -1e30); adding -1e30 in fp32 yields exactly -1e30.
  - exp and the row-sum are one ACT pass (activation accum_out); the row max
    is produced negated (reduce negate=True) to feed exp's bias directly.
  - probs are PE-transposed per 128x128 block; the context matmul runs over
    2-q-tile groups (N=256 moving) to amortize weight loads.
"""

import os
import sys

for _p in ("/root/.axon_site/_ro/trn_rl_repo", "/opt/trn_rl_repo"):
    if os.path.isdir(_p) and _p not in sys.path:
        sys.path.insert(0, _p)

import numpy as np

import concourse.bacc as bacc
import concourse.tile as tile
from concourse import mybir
from concourse.bass_utils import run_bass_kernel_spmd
from concourse.masks import make_identity

B, Q, KV, H, NH, HD = 2, 2048, 2048, 1024, 16, 64
NCORES = 8
HPC = NH // 4  # heads per core = 4
D = HPC * HD  # per-core hidden slice = 256
F32 = mybir.dt.float32
NEG = -1e30

QT = Q // 128  # 16 q tiles
KT = KV // 128  # 16 kv tiles
HT = H // 128  # 8 hidden k-tiles
QG = 2         # q tiles per context group (ctx moving N = QG*128)


def _enable_ldw_opt():
    import concourse.bass_utils as _bu
    if getattr(_bu, "_ldw_opt_patched", False):
        return
    _orig = _bu.run_command

    def _patched(argv, **kw):
        argv = [a.replace("--enable-ldw-opt=false", "--enable-ldw-opt=true")
                if isinstance(a, str) else a for a in argv]
        return _orig(argv, **kw)

    _bu.run_command = _patched
    _bu._ldw_opt_patched = True


def _build_program():
    if os.environ.get("K_LDWOPT"):
        _enable_ldw_opt()
    nc = bacc.Bacc("TRN2", target_bir_lowering=False, debug=False,
                   num_devices=NCORES)

    def din(name, shape):
        return nc.dram_tensor(name, shape, F32, kind="ExternalInput").ap()

    def dout(name, shape):
        return nc.dram_tensor(name, shape, F32, kind="ExternalOutput").ap()

    hidT = din("hidT", [H, Q])
    encT = din("encT", [H, KV])
    wq = din("wq", [H, D])
    wk = din("wk", [H, D])
    wv = din("wv", [H, D])
    bqh = din("bqh", [HD, HPC])    # per-head bias columns [64, 4]
    bkh = din("bkh", [HD, HPC])
    bv = din("bv", [1, D])
    wd = din("wd", [D, H])
    cosq = din("cosq", [HD, Q])      # cos(q_freqs).T / 8
    sinq = din("sinq", [HD, Q])      # swapped/signed sin(q_freqs).T / 8
    cosk = din("cosk", [HD, KV])
    sink = din("sink", [HD, KV])
    maskrow = din("maskrow", [1, KV])  # 0 or -1e30 per kv position

    q_outT = dout("q_outT", [HPC, HD, Q])
    k_outT = dout("k_outT", [HPC, HD, KV])
    v_out = dout("v_out", [KV, D])
    scores_out = dout("scores_out", [HPC, Q, KV])
    probs_out = dout("probs_out", [HPC, Q, KV])
    ctxT_out = dout("ctxT_out", [QT // QG, 2, 128, QG * 128])
    emb_out = dout("emb_out", [Q, H])

    with tile.TileContext(nc) as tc:
        with tc.tile_pool(name="res", bufs=1) as res:
            ident = res.tile([128, 128], F32)
            make_identity(nc, ident)
            vres = res.tile([128, KT, D], F32)          # v[kv, d], kv-tiled
            kaug = res.tile([HD + 1, HPC, KV], F32)     # k_rot^T + mask row
            qaug = res.tile([HD + 1, HPC, Q], F32)      # q_rot^T/8 + ones row

            # ======== stages A+B: projections + RoPE, interleaved ======
            # One loop over 256-wide sequence chunks; encoder (kT, v) and
            # q-side work alternate, and RoPE runs per-chunk on DVE/ACT so
            # it hides behind the projection matmuls on PE.
            for h in range(HPC):
                nc.sync.dma_start(out=kaug[HD:HD + 1, h, :], in_=maskrow)
                nc.vector.memset(qaug[HD:HD + 1, h, :], 1.0)
            CH = 256  # chunk width
            NCH = KV // CH
            with tc.tile_pool(name="projw", bufs=1) as projw:
                wq_sb = projw.tile([128, HT, D], F32)
                nc.sync.dma_start(out=wq_sb,
                                  in_=wq.rearrange("(t p) n -> p t n", p=128))
                wk_sb = projw.tile([128, HT, D], F32)
                nc.sync.dma_start(out=wk_sb,
                                  in_=wk.rearrange("(t p) n -> p t n", p=128))
                wv_sb = projw.tile([128, HT, D], F32)
                nc.sync.dma_start(out=wv_sb,
                                  in_=wv.rearrange("(t p) n -> p t n", p=128))
                bqh_sb = projw.tile([HD, HPC], F32)
                nc.sync.dma_start(out=bqh_sb, in_=bqh)
                bkh_sb = projw.tile([HD, HPC], F32)
                nc.sync.dma_start(out=bkh_sb, in_=bkh)
                bv_bc = projw.tile([128, D], F32)
                nc.gpsimd.dma_start(out=bv_bc, in_=bv.to_broadcast([128, D]))

                with tc.tile_pool(name="seq", bufs=3) as seq, \
                     tc.tile_pool(name="prj", bufs=2) as prj, \
                     tc.tile_pool(name="trg", bufs=2) as trg, \
                     tc.tile_pool(name="rtmp", bufs=2) as rtmp, \
                     tc.tile_pool(name="pab", bufs=2, space="PSUM") as pab:

                    def side(it, xT, w_sb, bh_sb, outT, aug, cosd, sind,
                             ctag, do_v):
                        sl = slice(it * CH, (it + 1) * CH)
                        xq = seq.tile([128, HT, CH], F32, tag="xq")
                        nc.sync.dma_start(
                            out=xq,
                            in_=xT[:, sl].rearrange("(t p) n -> p t n", p=128))
                        tq = prj.tile([HD, HPC, CH], F32, tag=ctag)
                        for dh in range(2):
                            pps = pab.tile([128, CH], F32, tag="pps")
                            for j in range(HT):
                                nc.tensor.matmul(
                                    pps,
                                    lhsT=w_sb[:, j, dh * 128:(dh + 1) * 128],
                                    rhs=xq[:, j, :],
                                    start=(j == 0), stop=(j == HT - 1))
                            for hh in range(2):
                                h = 2 * dh + hh
                                nc.scalar.add(
                                    tq[:, h, :],
                                    pps[hh * HD:(hh + 1) * HD, :],
                                    add=bh_sb[:, h:h + 1])
                        if do_v:
                            for s in range(2):
                                kvt = it * 2 + s
                                vps = pab.tile([128, D], F32, tag="vps")
                                for j in range(HT):
                                    nc.tensor.matmul(
                                        vps,
                                        lhsT=xq[:, j, s * 128:(s + 1) * 128],
                                        rhs=wv_sb[:, j, :],
                                        start=(j == 0), stop=(j == HT - 1))
                                nc.vector.tensor_add(vres[:, kvt, :], vps,
                                                     bv_bc)
                                nc.sync.dma_start(
                                    out=v_out[kvt * 128:(kvt + 1) * 128, :],
                                    in_=vres[:, kvt, :])
                        for h in range(HPC):
                            nc.sync.dma_start(out=outT[h, :, sl],
                                              in_=tq[:, h, :])
                        cc = trg.tile([HD, CH], F32, tag=ctag + "c")
                        nc.sync.dma_start(out=cc, in_=cosd[:, sl])
                        sc = trg.tile([HD, CH], F32, tag=ctag + "s")
                        nc.sync.dma_start(out=sc, in_=sind[:, sl])
                        for h in range(HPC):
                            tA = rtmp.tile([HD, CH], F32, tag=ctag + "A")
                            tC = rtmp.tile([HD, CH], F32, tag=ctag + "C")
                            nc.vector.tensor_mul(tA, tq[:, h, :], cc)
                            nc.vector.tensor_mul(tC, tq[:, h, :], sc)
                            nc.scalar.copy(aug[0:32, h, sl], tC[32:64, :])
                            nc.scalar.copy(aug[32:64, h, sl], tC[0:32, :])
                            nc.vector.tensor_add(aug[0:HD, h, sl],
                                                 aug[0:HD, h, sl], tA)

                    for it in range(NCH):
                        side(it, encT, wk_sb, bkh_sb, k_outT, kaug,
                             cosk, sink, "k", True)
                        side(it, hidT, wq_sb, bqh_sb, q_outT, qaug,
                             cosq, sinq, "q", False)

            # ======== stage C: attention ===============================
            NCG = 128 * QG  # context-group q width
            with tc.tile_pool(name="wdp", bufs=1) as wdp, \
                 tc.tile_pool(name="sc", bufs=2) as scp, \
                 tc.tile_pool(name="stat", bufs=6) as statp, \
                 tc.tile_pool(name="pr", bufs=1) as prp, \
                 tc.tile_pool(name="cx", bufs=2) as cxp, \
                 tc.tile_pool(name="psc", bufs=2, space="PSUM") as psc, \
                 tc.tile_pool(name="ppt", bufs=2, space="PSUM") as ppt, \
                 tc.tile_pool(name="pcx", bufs=2, space="PSUM") as pcx:
                wd_sb = wdp.tile([128, 2, H], F32)
                nc.sync.dma_start(out=wd_sb,
                                  in_=wd.rearrange("(t p) n -> p t n", p=128))
                for qg in range(QT // QG):
                    probsT = prp.tile([128, 2, KT, NCG], F32, tag="pT")
                    ctxT_a = cxp.tile([128, NCG], F32, tag="cta")
                    ctxT_b = cxp.tile([128, NCG], F32, tag="ctb")
                    for h in range(HPC):
                        hp, hh = divmod(h, 2)
                        for sq in range(QG):
                            qt = qg * QG + sq
                            scores_sb = scp.tile([128, KV], F32, tag="ssb")
                            negmax = statp.tile([128, 1], F32, tag="nm")
                            for c in range(2):
                                sc_ps = psc.tile([128, KV // 2], F32,
                                                 tag="scps")
                                for j in range(2):
                                    off = c * (KV // 2) + j * 512
                                    nc.tensor.matmul(
                                        sc_ps[:, j * 512:(j + 1) * 512],
                                        lhsT=qaug[:, h,
                                                  qt * 128:(qt + 1) * 128],
                                        rhs=kaug[:, h, off:off + 512],
                                        start=True, stop=True)
                                nc.scalar.copy(
                                    scores_sb[:,
                                              c * (KV // 2):(c + 1) * (KV // 2)],
                                    sc_ps)
                            nc.sync.dma_start(
                                out=scores_out[h, qt * 128:(qt + 1) * 128, :],
                                in_=scores_sb)
                            nc.vector.reduce_max(
                                out=negmax, in_=scores_sb,
                                axis=mybir.AxisListType.X, negate=True)
                            probs_sb = scp.tile([128, KV], F32, tag="psb")
                            rowsum = statp.tile([128, 1], F32, tag="rs")
                            nc.scalar.activation(
                                probs_sb, scores_sb,
                                mybir.ActivationFunctionType.Exp,
                                bias=negmax, scale=1.0, accum_out=rowsum)
                            recip = statp.tile([128, 1], F32, tag="rc")
                            nc.vector.reciprocal(recip, rowsum)
                            nc.vector.tensor_scalar_mul(probs_sb, probs_sb,
                                                        recip)
                            nc.sync.dma_start(
                                out=probs_out[h, qt * 128:(qt + 1) * 128, :],
                                in_=probs_sb)
                            for g4 in range(4):
                                ptp = ppt.tile([128, 512], F32, tag="ptp")
                                for i in range(4):
                                    kt = 4 * g4 + i
                                    nc.tensor.transpose(
                                        ptp[:, i * 128:(i + 1) * 128],
                                        probs_sb[:, kt * 128:(kt + 1) * 128],
                                        ident)
                                nc.scalar.copy(
                                    probsT[:, hh, 4 * g4:4 * g4 + 4,
                                           sq * 128:(sq + 1) * 128],
                                    ptp.rearrange("p (k n) -> p k n", n=128))
                        ctx_ps = pcx.tile([HD, NCG], F32, tag="cxps")
                        for kt in range(KT):
                            nc.tensor.matmul(
                                ctx_ps,
                                lhsT=vres[:, kt, h * HD:(h + 1) * HD],
                                rhs=probsT[:, hh, kt, :],
                                start=(kt == 0), stop=(kt == KT - 1))
                        dst = ctxT_a if h < 2 else ctxT_b
                        nc.scalar.copy(dst[(h % 2) * HD:(h % 2 + 1) * HD, :],
                                       ctx_ps)
                    nc.sync.dma_start(out=ctxT_out[qg, 0], in_=ctxT_a)
                    nc.sync.dma_start(out=ctxT_out[qg, 1], in_=ctxT_b)
                    # partial dense projection
                    for sq in range(QG):
                        qt = qg * QG + sq
                        emb_sb = cxp.tile([128, H], F32, tag="emb")
                        for n in range(2):
                            emb_ps = ppt.tile([128, 512], F32, tag="ptp")
                            nc.tensor.matmul(
                                emb_ps,
                                lhsT=ctxT_a[:, sq * 128:(sq + 1) * 128],
                                rhs=wd_sb[:, 0, n * 512:(n + 1) * 512],
                                start=True, stop=False)
                            nc.tensor.matmul(
                                emb_ps,
                                lhsT=ctxT_b[:, sq * 128:(sq + 1) * 128],
                                rhs=wd_sb[:, 1, n * 512:(n + 1) * 512],
                                start=False, stop=True)
                            nc.scalar.copy(emb_sb[:, n * 512:(n + 1) * 512],
                                           emb_ps)
                        nc.sync.dma_start(
                            out=emb_out[qt * 128:(qt + 1) * 128, :],
                            in_=emb_sb)
    nc.compile()
    return nc


def _host_prep(inputs):
    """Build the 8 per-core input maps."""
    hs = np.asarray(inputs["hidden_states"], dtype=np.float32)
    es = np.asarray(inputs["encoder_hidden_states"], dtype=np.float32)
    am = np.asarray(inputs["attention_mask"], dtype=np.float32)
    qf = np.asarray(inputs["q_freqs"], dtype=np.float32)
    kf = np.asarray(inputs["k_freqs"], dtype=np.float32)
    Wq = np.asarray(inputs["Wq"], dtype=np.float32)
    Wk = np.asarray(inputs["Wk"], dtype=np.float32)
    Wv = np.asarray(inputs["Wv"], dtype=np.float32)
    Wd = np.asarray(inputs["Wd"], dtype=np.float32)

    # RoPE: rot[d] = -x[d+32] (d<32), x[d-32] (d>=32); we compute
    # tC[d] = x[d] * s2[d] with s2[d] = ssin[(d+32)%64] (ssin = sign*sin),
    # then partition-rotate tC.
    scale = np.float32(1.0 / np.sqrt(HD))

    def s2_of(freqs):
        s = np.sin(freqs)
        ssin = s.copy()
        ssin[:, :HD // 2] *= -1.0
        return np.concatenate([ssin[:, HD // 2:], ssin[:, :HD // 2]], axis=1)

    cosq = np.ascontiguousarray((np.cos(qf) * scale).T)
    sinq = np.ascontiguousarray((s2_of(qf) * scale).T)
    cosk = np.ascontiguousarray(np.cos(kf).T)
    sink = np.ascontiguousarray(s2_of(kf).T)

    hsT = [np.ascontiguousarray(hs[b].T) for b in range(B)]
    esT = [np.ascontiguousarray(es[b].T) for b in range(B)]
    maskrows = [
        np.where(am[b, 0, 0] > 0.5, np.float32(0.0),
                 np.float32(NEG)).astype(np.float32)[None, :]
        for b in range(B)
    ]

    in_maps = []
    for c in range(NCORES):
        b, g = divmod(c, 4)
        cols = slice(g * D, (g + 1) * D)
        bq_c = np.asarray(inputs["bq"], dtype=np.float32)[cols]
        bk_c = np.asarray(inputs["bk"], dtype=np.float32)[cols]
        in_maps.append({
            "hidT": hsT[b],
            "encT": esT[b],
            "wq": np.ascontiguousarray(Wq[:, cols]),
            "wk": np.ascontiguousarray(Wk[:, cols]),
            "wv": np.ascontiguousarray(Wv[:, cols]),
            "bqh": np.ascontiguousarray(bq_c.reshape(HPC, HD).T),
            "bkh": np.ascontiguousarray(bk_c.reshape(HPC, HD).T),
            "bv": np.ascontiguousarray(
                np.asarray(inputs["bv"], dtype=np.float32)[None, cols]),
            "wd": np.ascontiguousarray(Wd[cols, :]),
            "cosq": cosq, "sinq": sinq, "cosk": cosk, "sink": sink,
            "maskrow": maskrows[b],
        })
    return in_maps


_PROGRAM_CACHE = {}


def _get_program():
    if "nc" not in _PROGRAM_CACHE:
        _PROGRAM_CACHE["nc"] = _build_program()
    return _PROGRAM_CACHE["nc"]


def kernel(trace=False, **inputs):
    nc = _get_program()
    in_maps = _host_prep(inputs)
    kw = dict(trace=True) if trace else {}
    br = run_bass_kernel_spmd(nc, in_maps, list(range(NCORES)), **kw)
    res = br.results

    hs = np.asarray(inputs["hidden_states"], dtype=np.float32)
    bd = np.asarray(inputs["bd"], dtype=np.float32)

    q = np.empty((B, NH, Q, HD), np.float32)
    k = np.empty((B, NH, KV, HD), np.float32)
    v = np.empty((B, NH, KV, HD), np.float32)
    scores = np.empty((B, NH, Q, KV), np.float32)
    probs = np.empty((B, NH, Q, KV), np.float32)
    ctx = np.empty((B, Q, H), np.float32)
    emb = np.zeros((B, Q, H), np.float32)

    for c in range(NCORES):
        b, g = divmod(c, 4)
        hsl = slice(g * HPC, (g + 1) * HPC)
        r = res[c]
        q[b, hsl] = r["q_outT"].transpose(0, 2, 1)
        k[b, hsl] = r["k_outT"].transpose(0, 2, 1)
        v[b, hsl] = r["v_out"].reshape(KV, HPC, HD).transpose(1, 0, 2)
        scores[b, hsl] = r["scores_out"]
        probs[b, hsl] = r["probs_out"]
        ct = r["ctxT_out"]  # [groups, half, row, q]
        ctx[b, :, g * D:(g + 1) * D] = (
            ct.transpose(0, 3, 1, 2).reshape(Q, D))
        emb[b] += r["emb_out"]

    emb += bd[None, None, :] + hs

    out = (emb, q, v, k, probs, scores, ctx)
    if trace:
        return out, br
    return out


# revision 25
# speedup vs baseline: 1.1747x; 1.1747x over previous
"""Multi-head cross-attention (B=2, Q=KV=2048, H=1024, 16 heads x 64) on 8
Trainium2 NeuronCores via Bass/Tile.

Sharding: core c handles batch b = c//4 and head group g = c%4 (4 heads, 256
hidden columns). Every core is fully independent (no collectives): it
computes its heads' q/k/v projections, RoPE, masked scores, softmax, context,
and a partial dense projection (its 256 rows of Wd). The host sums the 4
dense partials per batch and adds bias + residual, and transposes the
q/k/context outputs that the device produces in PE-friendly layouts.

Device-side structure:
  - hidden/encoder states arrive pre-transposed (hidT/encT [H, S]); q^T and
    k^T projections are computed directly (weights stationary, hidT moving),
    so no PE transposes are needed on the input side.
  - mask + 1/sqrt(d) scale are folded into the score matmul: stationary is
    q_rot^T augmented with a row of ones, moving is k_rot^T augmented with a
    row of where(mask, 0, -1e30); adding -1e30 in fp32 yields exactly -1e30.
  - exp and the row-sum are one ACT pass (activation accum_out); the row max
    is produced negated (reduce negate=True) to feed exp's bias directly.
  - probs are PE-transposed per 128x128 block; the context matmul runs over
    2-q-tile groups (N=256 moving) to amortize weight loads.
"""

import os
import sys

for _p in ("/root/.axon_site/_ro/trn_rl_repo", "/opt/trn_rl_repo"):
    if os.path.isdir(_p) and _p not in sys.path:
        sys.path.insert(0, _p)

import numpy as np

import concourse.bacc as bacc
import concourse.tile as tile
from concourse import mybir
from concourse.bass_utils import run_bass_kernel_spmd
from concourse.masks import make_identity

B, Q, KV, H, NH, HD = 2, 2048, 2048, 1024, 16, 64
NCORES = 8
HPC = NH // 4  # heads per core = 4
D = HPC * HD  # per-core hidden slice = 256
F32 = mybir.dt.float32
NEG = -1e30

QT = Q // 128  # 16 q tiles
KT = KV // 128  # 16 kv tiles
HT = H // 128  # 8 hidden k-tiles
QG = 2         # q tiles per context group (ctx moving N = QG*128)


def _enable_ldw_opt():
    import concourse.bass_utils as _bu
    if getattr(_bu, "_ldw_opt_patched", False):
        return
    _orig = _bu.run_command

    def _patched(argv, **kw):
        argv = [a.replace("--enable-ldw-opt=false", "--enable-ldw-opt=true")
                if isinstance(a, str) else a for a in argv]
        return _orig(argv, **kw)

    _bu.run_command = _patched
    _bu._ldw_opt_patched = True


def _build_program():
    if os.environ.get("K_LDWOPT"):
        _enable_ldw_opt()
    nc = bacc.Bacc("TRN2", target_bir_lowering=False, debug=False,
                   num_devices=NCORES)

    def din(name, shape):
        return nc.dram_tensor(name, shape, F32, kind="ExternalInput").ap()

    def dout(name, shape):
        return nc.dram_tensor(name, shape, F32, kind="ExternalOutput").ap()

    hidT = din("hidT", [H, Q])
    encT = din("encT", [H, KV])
    wq = din("wq", [H, D])
    wk = din("wk", [H, D])
    wv = din("wv", [H, D])
    bqh = din("bqh", [HD, HPC])    # per-head bias columns [64, 4]
    bkh = din("bkh", [HD, HPC])
    bv = din("bv", [1, D])
    wd = din("wd", [D, H])
    cosq = din("cosq", [HD, Q])      # cos(q_freqs).T / 8
    sinq = din("sinq", [HD, Q])      # swapped/signed sin(q_freqs).T / 8
    cosk = din("cosk", [HD, KV])
    sink = din("sink", [HD, KV])
    maskrow = din("maskrow", [1, KV])  # 0 or -1e30 per kv position

    q_outT = dout("q_outT", [HPC, HD, Q])
    k_outT = dout("k_outT", [HPC, HD, KV])
    v_out = dout("v_out", [KV, D])
    scores_out = dout("scores_out", [HPC, Q, KV])
    probs_out = dout("probs_out", [HPC, Q, KV])
    ctxT_out = dout("ctxT_out", [QT // QG, 2, 128, QG * 128])
    emb_out = dout("emb_out", [Q, H])

    with tile.TileContext(nc) as tc:
        with tc.tile_pool(name="res", bufs=1) as res:
            ident = res.tile([128, 128], F32)
            make_identity(nc, ident)
            vres = res.tile([128, KT, D], F32)          # v[kv, d], kv-tiled
            kaug = res.tile([HD + 1, HPC, KV], F32)     # k_rot^T + mask row
            qaug = res.tile([HD + 1, HPC, Q], F32)      # q_rot^T/8 + ones row

            # ======== stages A+B: projections + RoPE, interleaved ======
            # One loop over 256-wide sequence chunks; encoder (kT, v) and
            # q-side work alternate, and RoPE runs per-chunk on DVE/ACT so
            # it hides behind the projection matmuls on PE.
            for h in range(HPC):
                nc.sync.dma_start(out=kaug[HD:HD + 1, h, :], in_=maskrow)
                nc.vector.memset(qaug[HD:HD + 1, h, :], 1.0)
            CH = 512  # chunk width
            NCH = KV // CH
            with tc.tile_pool(name="projw", bufs=1) as projw:
                wq_sb = projw.tile([128, HT, D], F32)
                nc.sync.dma_start(out=wq_sb,
                                  in_=wq.rearrange("(t p) n -> p t n", p=128))
                wk_sb = projw.tile([128, HT, D], F32)
                nc.sync.dma_start(out=wk_sb,
                                  in_=wk.rearrange("(t p) n -> p t n", p=128))
                wv_sb = projw.tile([128, HT, D], F32)
                nc.sync.dma_start(out=wv_sb,
                                  in_=wv.rearrange("(t p) n -> p t n", p=128))
                bqh_sb = projw.tile([HD, HPC], F32)
                nc.sync.dma_start(out=bqh_sb, in_=bqh)
                bkh_sb = projw.tile([HD, HPC], F32)
                nc.sync.dma_start(out=bkh_sb, in_=bkh)
                bv_bc = projw.tile([128, D], F32)
                nc.gpsimd.dma_start(out=bv_bc, in_=bv.to_broadcast([128, D]))

                with tc.tile_pool(name="seq", bufs=2) as seq, \
                     tc.tile_pool(name="prj", bufs=1) as prj, \
                     tc.tile_pool(name="trg", bufs=1) as trg, \
                     tc.tile_pool(name="rtmp", bufs=1) as rtmp, \
                     tc.tile_pool(name="pab", bufs=2, space="PSUM") as pab:

                    def side(it, xT, w_sb, bh_sb, outT, aug, cosd, sind,
                             ctag, do_v):
                        sl = slice(it * CH, (it + 1) * CH)
                        xq = seq.tile([128, HT, CH], F32, tag="xq")
                        nc.sync.dma_start(
                            out=xq,
                            in_=xT[:, sl].rearrange("(t p) n -> p t n", p=128))
                        tq = prj.tile([HD, HPC, CH], F32, tag=ctag)
                        for dh in range(2):
                            pps = pab.tile([128, CH], F32, tag="pps")
                            for j in range(HT):
                                nc.tensor.matmul(
                                    pps,
                                    lhsT=w_sb[:, j, dh * 128:(dh + 1) * 128],
                                    rhs=xq[:, j, :],
                                    start=(j == 0), stop=(j == HT - 1))
                            for hh in range(2):
                                h = 2 * dh + hh
                                nc.scalar.add(
                                    tq[:, h, :],
                                    pps[hh * HD:(hh + 1) * HD, :],
                                    add=bh_sb[:, h:h + 1])
                        if do_v:
                            for s in range(CH // 128):
                                kvt = it * (CH // 128) + s
                                vps = pab.tile([128, D], F32, tag="vps")
                                for j in range(HT):
                                    nc.tensor.matmul(
                                        vps,
                                        lhsT=xq[:, j, s * 128:(s + 1) * 128],
                                        rhs=wv_sb[:, j, :],
                                        start=(j == 0), stop=(j == HT - 1))
                                nc.vector.tensor_add(vres[:, kvt, :], vps,
                                                     bv_bc)
                                nc.sync.dma_start(
                                    out=v_out[kvt * 128:(kvt + 1) * 128, :],
                                    in_=vres[:, kvt, :])
                        for h in range(HPC):
                            nc.sync.dma_start(out=outT[h, :, sl],
                                              in_=tq[:, h, :])
                        cc = trg.tile([HD, CH], F32, tag=ctag + "c")
                        nc.sync.dma_start(out=cc, in_=cosd[:, sl])
                        sc = trg.tile([HD, CH], F32, tag=ctag + "s")
                        nc.sync.dma_start(out=sc, in_=sind[:, sl])
                        for h in range(HPC):
                            tA = rtmp.tile([HD, CH], F32, tag=ctag + "A")
                            tC = rtmp.tile([HD, CH], F32, tag=ctag + "C")
                            nc.vector.tensor_mul(tA, tq[:, h, :], cc)
                            nc.vector.tensor_mul(tC, tq[:, h, :], sc)
                            nc.scalar.copy(aug[0:32, h, sl], tC[32:64, :])
                            nc.scalar.copy(aug[32:64, h, sl], tC[0:32, :])
                            nc.vector.tensor_add(aug[0:HD, h, sl],
                                                 aug[0:HD, h, sl], tA)

                    for it in range(NCH):
                        side(it, encT, wk_sb, bkh_sb, k_outT, kaug,
                             cosk, sink, "k", True)
                        side(it, hidT, wq_sb, bqh_sb, q_outT, qaug,
                             cosq, sinq, "q", False)

            # ======== stage C: attention ===============================
            NCG = 128 * QG  # context-group q width
            with tc.tile_pool(name="wdp", bufs=1) as wdp, \
                 tc.tile_pool(name="sc", bufs=2) as scp, \
                 tc.tile_pool(name="stat", bufs=6) as statp, \
                 tc.tile_pool(name="pr", bufs=1) as prp, \
                 tc.tile_pool(name="cx", bufs=2) as cxp, \
                 tc.tile_pool(name="psc", bufs=2, space="PSUM") as psc, \
                 tc.tile_pool(name="ppt", bufs=2, space="PSUM") as ppt, \
                 tc.tile_pool(name="pcx", bufs=2, space="PSUM") as pcx:
                wd_sb = wdp.tile([128, 2, H], F32)
                nc.sync.dma_start(out=wd_sb,
                                  in_=wd.rearrange("(t p) n -> p t n", p=128))
                for qg in range(QT // QG):
                    probsT = prp.tile([128, 2, KT, NCG], F32, tag="pT")
                    ctxT_a = cxp.tile([128, NCG], F32, tag="cta")
                    ctxT_b = cxp.tile([128, NCG], F32, tag="ctb")
                    for h in range(HPC):
                        hp, hh = divmod(h, 2)
                        for sq in range(QG):
                            qt = qg * QG + sq
                            scores_sb = scp.tile([128, KV], F32, tag="ssb")
                            negmax = statp.tile([128, 1], F32, tag="nm")
                            for c in range(2):
                                sc_ps = psc.tile([128, KV // 2], F32,
                                                 tag="scps")
                                for j in range(2):
                                    off = c * (KV // 2) + j * 512
                                    nc.tensor.matmul(
                                        sc_ps[:, j * 512:(j + 1) * 512],
                                        lhsT=qaug[:, h,
                                                  qt * 128:(qt + 1) * 128],
                                        rhs=kaug[:, h, off:off + 512],
                                        start=True, stop=True)
                                nc.scalar.copy(
                                    scores_sb[:,
                                              c * (KV // 2):(c + 1) * (KV // 2)],
                                    sc_ps)
                            nc.sync.dma_start(
                                out=scores_out[h, qt * 128:(qt + 1) * 128, :],
                                in_=scores_sb)
                            nc.vector.reduce_max(
                                out=negmax, in_=scores_sb,
                                axis=mybir.AxisListType.X, negate=True)
                            probs_sb = scp.tile([128, KV], F32, tag="psb")
                            rowsum = statp.tile([128, 1], F32, tag="rs")
                            nc.scalar.activation(
                                probs_sb, scores_sb,
                                mybir.ActivationFunctionType.Exp,
                                bias=negmax, scale=1.0, accum_out=rowsum)
                            recip = statp.tile([128, 1], F32, tag="rc")
                            nc.vector.reciprocal(recip, rowsum)
                            nc.vector.tensor_scalar_mul(probs_sb, probs_sb,
                                                        recip)
                            nc.sync.dma_start(
                                out=probs_out[h, qt * 128:(qt + 1) * 128, :],
                                in_=probs_sb)
                            for g4 in range(4):
                                ptp = ppt.tile([128, 512], F32, tag="ptp")
                                for i in range(4):
                                    kt = 4 * g4 + i
                                    nc.tensor.transpose(
                                        ptp[:, i * 128:(i + 1) * 128],
                                        probs_sb[:, kt * 128:(kt + 1) * 128],
                                        ident)
                                nc.scalar.copy(
                                    probsT[:, hh, 4 * g4:4 * g4 + 4,
                                           sq * 128:(sq + 1) * 128],
                                    ptp.rearrange("p (k n) -> p k n", n=128))
                        ctx_ps = pcx.tile([HD, NCG], F32, tag="cxps")
                        for kt in range(KT):
                            nc.tensor.matmul(
                                ctx_ps,
                                lhsT=vres[:, kt, h * HD:(h + 1) * HD],
                                rhs=probsT[:, hh, kt, :],
                                start=(kt == 0), stop=(kt == KT - 1))
                        dst = ctxT_a if h < 2 else ctxT_b
                        nc.scalar.copy(dst[(h % 2) * HD:(h % 2 + 1) * HD, :],
                                       ctx_ps)
                    nc.sync.dma_start(out=ctxT_out[qg, 0], in_=ctxT_a)
                    nc.sync.dma_start(out=ctxT_out[qg, 1], in_=ctxT_b)
                    # partial dense projection
                    for sq in range(QG):
                        qt = qg * QG + sq
                        emb_sb = cxp.tile([128, H], F32, tag="emb")
                        for n in range(2):
                            emb_ps = ppt.tile([128, 512], F32, tag="ptp")
                            nc.tensor.matmul(
                                emb_ps,
                                lhsT=ctxT_a[:, sq * 128:(sq + 1) * 128],
                                rhs=wd_sb[:, 0, n * 512:(n + 1) * 512],
                                start=True, stop=False)
                            nc.tensor.matmul(
                                emb_ps,
                                lhsT=ctxT_b[:, sq * 128:(sq + 1) * 128],
                                rhs=wd_sb[:, 1, n * 512:(n + 1) * 512],
                                start=False, stop=True)
                            nc.scalar.copy(emb_sb[:, n * 512:(n + 1) * 512],
                                           emb_ps)
                        nc.sync.dma_start(
                            out=emb_out[qt * 128:(qt + 1) * 128, :],
                            in_=emb_sb)
    nc.compile()
    return nc


def _host_prep(inputs):
    """Build the 8 per-core input maps."""
    hs = np.asarray(inputs["hidden_states"], dtype=np.float32)
    es = np.asarray(inputs["encoder_hidden_states"], dtype=np.float32)
    am = np.asarray(inputs["attention_mask"], dtype=np.float32)
    qf = np.asarray(inputs["q_freqs"], dtype=np.float32)
    kf = np.asarray(inputs["k_freqs"], dtype=np.float32)
    Wq = np.asarray(inputs["Wq"], dtype=np.float32)
    Wk = np.asarray(inputs["Wk"], dtype=np.float32)
    Wv = np.asarray(inputs["Wv"], dtype=np.float32)
    Wd = np.asarray(inputs["Wd"], dtype=np.float32)

    # RoPE: rot[d] = -x[d+32] (d<32), x[d-32] (d>=32); we compute
    # tC[d] = x[d] * s2[d] with s2[d] = ssin[(d+32)%64] (ssin = sign*sin),
    # then partition-rotate tC.
    scale = np.float32(1.0 / np.sqrt(HD))

    def s2_of(freqs):
        s = np.sin(freqs)
        ssin = s.copy()
        ssin[:, :HD // 2] *= -1.0
        return np.concatenate([ssin[:, HD // 2:], ssin[:, :HD // 2]], axis=1)

    cosq = np.ascontiguousarray((np.cos(qf) * scale).T)
    sinq = np.ascontiguousarray((s2_of(qf) * scale).T)
    cosk = np.ascontiguousarray(np.cos(kf).T)
    sink = np.ascontiguousarray(s2_of(kf).T)

    hsT = [np.ascontiguousarray(hs[b].T) for b in range(B)]
    esT = [np.ascontiguousarray(es[b].T) for b in range(B)]
    maskrows = [
        np.where(am[b, 0, 0] > 0.5, np.float32(0.0),
                 np.float32(NEG)).astype(np.float32)[None, :]
        for b in range(B)
    ]

    in_maps = []
    for c in range(NCORES):
        b, g = divmod(c, 4)
        cols = slice(g * D, (g + 1) * D)
        bq_c = np.asarray(inputs["bq"], dtype=np.float32)[cols]
        bk_c = np.asarray(inputs["bk"], dtype=np.float32)[cols]
        in_maps.append({
            "hidT": hsT[b],
            "encT": esT[b],
            "wq": np.ascontiguousarray(Wq[:, cols]),
            "wk": np.ascontiguousarray(Wk[:, cols]),
            "wv": np.ascontiguousarray(Wv[:, cols]),
            "bqh": np.ascontiguousarray(bq_c.reshape(HPC, HD).T),
            "bkh": np.ascontiguousarray(bk_c.reshape(HPC, HD).T),
            "bv": np.ascontiguousarray(
                np.asarray(inputs["bv"], dtype=np.float32)[None, cols]),
            "wd": np.ascontiguousarray(Wd[cols, :]),
            "cosq": cosq, "sinq": sinq, "cosk": cosk, "sink": sink,
            "maskrow": maskrows[b],
        })
    return in_maps


_PROGRAM_CACHE = {}


def _get_program():
    if "nc" not in _PROGRAM_CACHE:
        _PROGRAM_CACHE["nc"] = _build_program()
    return _PROGRAM_CACHE["nc"]


def kernel(trace=False, **inputs):
    nc = _get_program()
    in_maps = _host_prep(inputs)
    kw = dict(trace=True) if trace else {}
    br = run_bass_kernel_spmd(nc, in_maps, list(range(NCORES)), **kw)
    res = br.results

    hs = np.asarray(inputs["hidden_states"], dtype=np.float32)
    bd = np.asarray(inputs["bd"], dtype=np.float32)

    q = np.empty((B, NH, Q, HD), np.float32)
    k = np.empty((B, NH, KV, HD), np.float32)
    v = np.empty((B, NH, KV, HD), np.float32)
    scores = np.empty((B, NH, Q, KV), np.float32)
    probs = np.empty((B, NH, Q, KV), np.float32)
    ctx = np.empty((B, Q, H), np.float32)
    emb = np.zeros((B, Q, H), np.float32)

    for c in range(NCORES):
        b, g = divmod(c, 4)
        hsl = slice(g * HPC, (g + 1) * HPC)
        r = res[c]
        q[b, hsl] = r["q_outT"].transpose(0, 2, 1)
        k[b, hsl] = r["k_outT"].transpose(0, 2, 1)
        v[b, hsl] = r["v_out"].reshape(KV, HPC, HD).transpose(1, 0, 2)
        scores[b, hsl] = r["scores_out"]
        probs[b, hsl] = r["probs_out"]
        ct = r["ctxT_out"]  # [groups, half, row, q]
        ctx[b, :, g * D:(g + 1) * D] = (
            ct.transpose(0, 3, 1, 2).reshape(Q, D))
        emb[b] += r["emb_out"]

    emb += bd[None, None, :] + hs

    out = (emb, q, v, k, probs, scores, ctx)
    if trace:
        return out, br
    return out
